# revision 23
# baseline (speedup 1.0000x reference)
"""Policy-masked multi-head attention for Trainium2 (Bass/Tile), v2.

Full-input contract: kernel(**inputs) takes the complete tensors and returns
the complete (N, B, C) output. Internally data-parallel over the batch dim:
core b computes batch b on one NeuronCore (no collectives). Host-side prep
is layout only: per-core slicing, an x transpose, and bf16 casts of x,
W_qk, W_v, W_proj.

Per-core math (N=1024 tokens, C=768, H=12 heads, hd=64):
  qkv = x @ W_qkv + b_qkv ; per head: S = q k^T / 8
  A   = exp(S) * mask      (mask = p[key] off-diag, 1 on diag)
  out = (A @ v) / (sum_k A) ; y = out @ W_proj + b_proj
Softmax max-subtraction is skipped (exp args bounded); the ~1e-6-relative
eps regularizers are dropped (denominators are O(100)).

v2 structure (vs the v1 baseline; total rel err ~5.2e-3):
  - weights/biases/policy-derived constants live in the persist pool:
    loaded/computed once per NEFF, shared by every replicated body (the
    per-body DMA is just x in bf16 halves on two queues, plus y out)
  - the whole x / W_qk / W_v / W_proj path is bf16 (PE rate is the same
    as f32r, but DMA and SBUF halve and allow cross-body overlap)
  - the always-keep-diagonal policy fix is applied in PSUM by tiny PE
    matmuls (diag(-8 ln p) @ I accumulated onto the S tile, split into
    base-0 / base-64 variants so each accumulation group keeps a single
    lhsT base partition) instead of 96 DVE scalar_tensor_tensor ops --
    this keeps the S -> exp -> AV chain off the DVE
  - softmax denominators (free via the ones lane in the v layout) are
    partition-broadcast by GPSIMD, reciprocal'd on DVE, and the normalize
    multiply is fused with the PSUM->SBUF out copy (no PE broadcast
    matmuls, no eps)
  - PSUM: qk pool (2 banks) + S pair pool (4) + AV (2) = 8 banks; a
    matmul output must not cross a 2 KB PSUM bank, so every matmul
    writes a 512-f32 (or narrower) region

Hardware gotchas encoded here (from v1, found empirically):
  - float32r matmul operands must come from an f32r-dtype producer
  - gpsimd.partition_broadcast only works from AP base partition 0 to a
    full-partition destination
  - two matmuls with different lhsT base partitions must not share a psum
    accumulation group (device fault)
  - engine ops need 32-aligned base partitions
"""

import sys

if "/opt/trn_rl_repo" not in sys.path:
    sys.path.insert(0, "/opt/trn_rl_repo")

import numpy as np

N, B, C = 1024, 8, 768
NH = 12          # heads
HD = 64          # head dim
P = 128          # partitions
NT = N // P      # 8 token tiles
CT = C // P      # 6 channel tiles
CP = CT // 2     # 3 channel-tile pairs (fp8 DoubleRow)
HP = NH // 2     # 6 head pairs
SCALE = 0.125    # hd**-0.5
# per (tile, pair) v-lane layout: [vA(64) | 1 | 0(63) | vB(64)] (+pad)
# head A stationary window = cols [0:65)   -> psum rows 0:64 out, row 64 denom
# head B stationary window = cols [64:192) -> psum rows 64:128 out; window
#   col 0 is head A's ones lane, so psum row 0 = head B's denominator
VW = 192

_CACHE = {}


def _build(reps=1):
    if reps in _CACHE:
        return _CACHE[reps]

    from contextlib import ExitStack

    import concourse.bass as bass
    import concourse.tile as tile
    from concourse import bacc, mybir
    from concourse.masks import make_identity

    f32 = mybir.dt.float32
    f32r = mybir.dt.float32r
    bf16 = mybir.dt.bfloat16
    fp8 = mybir.dt.float8e4
    Act = mybir.ActivationFunctionType
    DR = mybir.MatmulPerfMode.DoubleRow

    nc = bacc.Bacc()

    xT_d = nc.declare_dram_parameter("xT", [C, N], bf16, isOutput=False)
    pol_d = nc.declare_dram_parameter("policy", [N], f32, isOutput=False)
    wqk_d = nc.declare_dram_parameter("W_qk", [C, 2 * C], bf16, isOutput=False)
    wv_d = nc.declare_dram_parameter("W_v", [C, C], bf16, isOutput=False)
    wproj_d = nc.declare_dram_parameter("Wp16", [C, C], bf16, isOutput=False)
    bqkv_d = nc.declare_dram_parameter("b_qkv", [3 * C], f32, isOutput=False)
    bproj_d = nc.declare_dram_parameter("b_proj", [C], f32, isOutput=False)
    y_d = nc.declare_dram_parameter("y", [N, C], f32, isOutput=True)

    xT_v = xT_d.rearrange("(ct p) n -> p ct n", p=P)           # (128, 6, 1024)
    pol_v = pol_d.rearrange("(t p) -> p t", p=P)               # (128, 8)
    wqk_v = wqk_d.rearrange("(ct p) j -> p ct j", p=P)         # (128, 6, 1536)
    wv_v = wv_d.rearrange("(ct p) j -> p ct j", p=P)           # (128, 6, 768)
    wproj_v = wproj_d.rearrange("(ct p) j -> p ct j", p=P)
    bqk_v = bqkv_d[0 : 2 * C].rearrange("(t p) -> p t", p=P)   # (128, 12)
    y_v = y_d.rearrange("(t p) c -> p t c", p=P)

    def bcast(dram_ap, parts):
        # partition-broadcast read of a 1-D dram slice (step-0 partition dim)
        return bass.AP(
            tensor=dram_ap.tensor, offset=dram_ap.offset, ap=[[0, parts]] + dram_ap.ap
        )

    with tile.TileContext(nc) as tc, ExitStack() as ctx:
        persist = ctx.enter_context(tc.tile_pool(name="persist", bufs=1))
        qkT_pool = ctx.enter_context(tc.tile_pool(name="qkT", bufs=4))
        A_pool = ctx.enter_context(tc.tile_pool(name="Apool", bufs=10))
        rb_pool = ctx.enter_context(tc.tile_pool(name="rbpool", bufs=2))
        d_pool = ctx.enter_context(tc.tile_pool(name="dpool", bufs=2))

        # ---- weights / biases (once per NEFF) ---------------------------
        wqk_sb = persist.tile([P, CT, 2 * C], bf16, name="wqk")    # 18 KB
        # Act's DGE queue: leaves sync free for the first body's x halves
        nc.scalar.dma_start(wqk_sb, wqk_v)
        wv_sb = persist.tile([P, CT, C], bf16, name="wv")          # 9 KB
        nc.gpsimd.dma_start(wv_sb, wv_v)
        wproj_sb = persist.tile([P, CT, C], bf16, name="wproj")    # 9 KB
        nc.gpsimd.dma_start(wproj_sb, wproj_v)
        bqk_sb = persist.tile([P, 2 * CT], f32)
        nc.sync.dma_start(bqk_sb, bqk_v)
        bv_bc = persist.tile([P, C], f32)
        nc.sync.dma_start(bv_bc, bcast(bqkv_d[2 * C : 3 * C], P))
        bproj_bc = persist.tile([P, C], f32)
        nc.sync.dma_start(bproj_bc, bcast(bproj_d[:], P))

        # ---- policy-derived constants (once per NEFF) -------------------
        ident = persist.tile([P, P], f32)
        make_identity(nc, ident)
        identB = persist.tile([P, P], bf16)
        nc.vector.tensor_copy(identB, ident)
        # identSh rows 64:128 = I[0:64]  (for the base-64 diag fix, k < 64)
        identSh = persist.tile([P, P], bf16)
        nc.vector.tensor_copy(identSh[HD:P, :], identB[0:HD, :])

        pol_sb = persist.tile([P, NT], f32)
        nc.sync.dma_start(pol_sb, pol_v)
        logp = persist.tile([P, NT], f32)
        # clamp away exact zeros so ln() stays finite (ln(1e-38) = -87.5)
        nc.vector.tensor_scalar_max(logp, pol_sb, 1e-38)
        nc.scalar.activation(logp, logp, Act.Ln)
        n8logp = persist.tile([P, NT], f32)
        nc.vector.tensor_scalar_mul(n8logp, logp, -8.0)
        # D_full[:, t, :] = diag(-8 ln p) for key tile t (bf16, lhsT base 0)
        D_full = persist.tile([P, NT, P], bf16)
        for t in range(NT):
            nc.vector.tensor_scalar_mul(D_full[:, t, :], identB, n8logp[:, t : t + 1])
        # Dsh: base-64 diag-fix stationaries, rows 64:128:
        #   [:, t, 0:128]   = D_full[0:64, t, :]   (diag cells k < 64)
        #   [:, t, 128:256] = D_full[64:128, t, :] (diag cells k >= 64)
        Dsh = persist.tile([P, NT, 2 * P], bf16)
        for t in range(NT):
            nc.vector.tensor_copy(Dsh[HD:P, t, 0:P], D_full[0:HD, t, :])
            nc.vector.tensor_copy(Dsh[HD:P, t, P : 2 * P], D_full[HD:P, t, :])

        def emit_body(rep):
            body = ExitStack()
            bpool = body.enter_context(tc.tile_pool(name=f"body{rep}", bufs=1))

            # ---- activations in, consumption order ----------------------
            # split x into halves on separate DGE queues: the second half
            # (and the next body's first half) overlaps compute
            xTa = bpool.tile([P, CT // 2, N], bf16, name="xTa")  # 6 KB
            nc.sync.dma_start(xTa, xT_v[:, 0 : CT // 2, :])
            xTb = bpool.tile([P, CT // 2, N], bf16, name="xTb")  # 6 KB
            nc.gpsimd.dma_start(xTb, xT_v[:, CT // 2 : CT, :])

            def xt(ct):
                return xTa[:, ct, :] if ct < CT // 2 else xTb[:, ct - CT // 2, :]

            vv = bpool.tile([P, NT, HP, VW], bf16, name="vv")   # 18 KB
            outT = bpool.tile([P, HP, N], bf16, name="outT")    # 12 KB

            # ones/zeros lanes of the v layout
            nc.gpsimd.memset(vv[:, :, :, HD : 2 * HD], 0.0)
            nc.gpsimd.memset(vv[:, :, :, HD : HD + 1], 1.0)

            # ---- phase 1: v projection ----------------------------------
            with tc.tile_pool(name=f"psv{rep}", bufs=3, space="PSUM") as psv:
                for t in range(NT):
                    pv = psv.tile([P, C], f32, tag="pv")
                    for ct in range(CT):
                        lhs = xt(ct)[:, t * P : (t + 1) * P]
                        nc.tensor.matmul(
                            pv[:, 0:512], lhs, wv_sb[:, ct, 0:512],
                            start=(ct == 0), stop=(ct == CT - 1),
                        )
                        nc.tensor.matmul(
                            pv[:, 512:768], lhs, wv_sb[:, ct, 512:768],
                            start=(ct == 0), stop=(ct == CT - 1),
                        )
                    pv3 = pv.rearrange("p (hp w) -> p hp w", w=P)
                    bv3 = bv_bc.rearrange("p (hp w) -> p hp w", w=P)
                    nc.vector.tensor_add(
                        vv[:, t, :, 0:HD], pv3[:, :, 0:HD], bv3[:, :, 0:HD]
                    )
                    nc.vector.tensor_add(
                        vv[:, t, :, P : P + HD], pv3[:, :, HD:P], bv3[:, :, HD:P]
                    )

            # ---- phase 2: per head-pair: qk -> S(+diag) -> exp -> AV ----
            with tc.tile_pool(
                name=f"psqk{rep}", bufs=1, space="PSUM"
            ) as psqk, tc.tile_pool(
                name=f"ps{rep}", bufs=2, space="PSUM"
            ) as ps, tc.tile_pool(name=f"psav{rep}", bufs=1, space="PSUM") as psav:
                for hp in range(HP):
                    # --- kT, qT for this pair (two 128-row j-tiles) ---
                    # kT first: its copy-out overlaps the qT matmuls; both
                    # copies split in halves so S can start off the first
                    pair_qk = []
                    for sec, jt in ((1, CT + hp), (0, hp)):  # k tile, q tile
                        colbase = sec * C + hp * P
                        pq = psqk.tile([P, N], f32, tag="pq", name=f"pq{sec}")
                        for ct in range(CT):
                            for h in range(2):
                                nc.tensor.matmul(
                                    pq[:, h * 512 : (h + 1) * 512],
                                    wqk_sb[:, ct, colbase : colbase + P],
                                    xt(ct)[:, h * 512 : (h + 1) * 512],
                                    start=(ct == 0), stop=(ct == CT - 1),
                                )
                        dst = qkT_pool.tile([P, N], bf16, tag="qkT")
                        for h in range(2):
                            sl = slice(h * 512, (h + 1) * 512)
                            nc.vector.tensor_scalar_add(
                                dst[:, sl], pq[:, sl], bqk_sb[:, jt : jt + 1]
                            )
                        pair_qk.append(dst)
                    kT_t, qT_t = pair_qk

                    # --- S_T (+ PE diag fix) + masked exp, per key tile ---
                    A_t = [None, None]  # [hi][tk] -> bf16 [128, 1024]
                    A_t[0] = [None] * NT
                    A_t[1] = [None] * NT
                    for tk in range(NT):
                        hdq = tk * P  # column of this tile's diagonal block
                        # interleave the two heads' matmuls so adjacent MMs
                        # sit in disjoint PE row groups (0:64 vs 64:128) and
                        # run concurrently in the array
                        dh = tk // 4  # which 512-half holds the diag block
                        pss_pair = [
                            ps.tile([P, N], f32, tag="s", name=f"ps{i}")
                            for i in range(2)
                        ]
                        for h in range(2):
                            for hi, o in ((0, 0), (1, HD)):
                                nc.tensor.matmul(
                                    pss_pair[hi][:, h * 512 : (h + 1) * 512],
                                    kT_t[o : o + HD, tk * P : (tk + 1) * P],
                                    qT_t[o : o + HD, h * 512 : (h + 1) * 512],
                                    start=True, stop=(h != dh),
                                )
                        # always-keep diagonal: S += diag(-8 ln p) via tiny
                        # PE matmuls; lhsT base must match the S matmul's
                        # (base 0 for head A, base 64 for head B)
                        nc.tensor.matmul(
                            pss_pair[0][:, hdq : hdq + P],
                            D_full[:, tk, :], identB,
                            start=False, stop=True,
                        )
                        nc.tensor.matmul(
                            pss_pair[1][:, hdq : hdq + P],
                            Dsh[HD:P, tk, 0:P], identSh[HD:P, :],
                            start=False, stop=False,
                        )
                        nc.tensor.matmul(
                            pss_pair[1][:, hdq : hdq + P],
                            Dsh[HD:P, tk, P : 2 * P], identB[HD:P, :],
                            start=False, stop=True,
                        )
                        for hi in range(2):
                            pss = pss_pair[hi]
                            At = A_pool.tile([P, N], bf16, tag="A")
                            # A = exp(s/8 + ln p_key); diag pre-biased so the
                            # ln p cancels there (always-keep)
                            nc.scalar.activation(
                                At, pss, Act.Exp,
                                bias=logp[:, tk : tk + 1], scale=SCALE,
                            )
                            A_t[hi][tk] = At

                    # --- AV + denominator + normalize, per head ---
                    for hi in range(2):
                        pav = psav.tile([P, N], f32, tag="av")
                        if hi == 0:
                            vsl = (0, HD + 1)   # [vA | 1] -> rows 0:64, 64=den
                            orows, drow = (0, HD), HD
                        else:
                            vsl = (HD, HD + P)  # -> rows 64:128 out, 0 = den
                            orows, drow = (HD, P), 0
                        mrows = vsl[1] - vsl[0]
                        for tk in range(NT):
                            for h in range(2):
                                nc.tensor.matmul(
                                    pav[:mrows, h * 512 : (h + 1) * 512],
                                    vv[:, tk, hp, vsl[0] : vsl[1]],
                                    A_t[hi][tk][:, h * 512 : (h + 1) * 512],
                                    start=(tk == 0), stop=(tk == NT - 1),
                                )
                        # denominator row -> sbuf -> broadcast -> 1/x
                        dd = d_pool.tile([1, N], f32, tag="d")
                        nc.vector.tensor_copy(dd, pav[drow : drow + 1, :])
                        rb = rb_pool.tile([P, N], f32, tag="rb")
                        nc.gpsimd.partition_broadcast(rb, dd[0:1, :])
                        nc.vector.reciprocal(
                            rb[orows[0] : orows[1], :], rb[orows[0] : orows[1], :]
                        )
                        # normalize fused with the PSUM->SBUF copy
                        nc.vector.tensor_mul(
                            outT[orows[0] : orows[1], hp, :],
                            pav[orows[0] : orows[1], :],
                            rb[orows[0] : orows[1], :],
                        )

            # ---- phase 3: output projection -----------------------------
            with tc.tile_pool(
                name=f"psy{rep}", bufs=3, space="PSUM"
            ) as psy, tc.tile_pool(name=f"yout{rep}", bufs=4) as yp:
                for t in range(NT):
                    py = psy.tile([P, C], f32, tag="y")
                    for ct in range(CT):
                        nc.tensor.matmul(
                            py[:, 0:512],
                            outT[:, ct, t * P : (t + 1) * P],
                            wproj_sb[:, ct, 0:512],
                            start=(ct == 0), stop=(ct == CT - 1),
                        )
                        nc.tensor.matmul(
                            py[:, 512:768],
                            outT[:, ct, t * P : (t + 1) * P],
                            wproj_sb[:, ct, 512:768],
                            start=(ct == 0), stop=(ct == CT - 1),
                        )
                    y_sb = yp.tile([P, C], f32, tag="yo")
                    nc.vector.tensor_add(y_sb, py, bproj_bc)
                    nc.gpsimd.dma_start(y_v[:, t, :], y_sb)
            body.close()

        for rep in range(reps):
            emit_body(rep)

    nc.finalize()
    _CACHE[reps] = nc
    return nc


def make_in_maps(x, policy, W_qkv, b_qkv, W_proj, b_proj):
    import ml_dtypes

    fp8 = ml_dtypes.float8_e4m3

    x = np.asarray(x, dtype=np.float32)           # (N, B, C)
    policy = np.asarray(policy, dtype=np.float32).reshape(B, N)
    W_qkv = np.asarray(W_qkv, dtype=np.float32)   # (C, 3C)
    W_proj = np.asarray(W_proj, dtype=np.float32)

    bfl = ml_dtypes.bfloat16
    shared = {
        "W_qk": np.ascontiguousarray(W_qkv[:, 0 : 2 * C].astype(bfl)),
        "W_v": np.ascontiguousarray(W_qkv[:, 2 * C : 3 * C].astype(bfl)),
        "Wp16": np.ascontiguousarray(W_proj.astype(ml_dtypes.bfloat16)),
        "b_qkv": np.ascontiguousarray(np.asarray(b_qkv, dtype=np.float32)),
        "b_proj": np.ascontiguousarray(np.asarray(b_proj, dtype=np.float32)),
    }
    maps = []
    for b in range(B):
        xT = np.ascontiguousarray(x[:, b, :].T.astype(bfl))  # (C, N)
        maps.append({
            "xT": xT,
            "policy": np.ascontiguousarray(policy[b]),
            **shared,
        })
    return maps


def kernel(x, policy, W_qkv, b_qkv, W_proj, b_proj):
    from concourse.bass_utils import run_bass_kernel_spmd

    nc = _build()
    in_maps = make_in_maps(x, policy, W_qkv, b_qkv, W_proj, b_proj)
    res = run_bass_kernel_spmd(nc, in_maps, core_ids=list(range(B)))
    y = np.stack([res.results[i]["y"] for i in range(B)], axis=1)  # (N, B, C)
    return np.ascontiguousarray(y.astype(np.float32))


# revision 25
# speedup vs baseline: 1.2666x; 1.2666x over previous
"""Policy-masked multi-head attention for Trainium2 (Bass/Tile), v2.

Full-input contract: kernel(**inputs) takes the complete tensors and returns
the complete (N, B, C) output. Internally data-parallel over the batch dim:
core b computes batch b on one NeuronCore (no collectives). Host-side prep
is layout only: per-core slicing, an x transpose, and bf16 casts of x,
W_qk, W_v, W_proj.

Per-core math (N=1024 tokens, C=768, H=12 heads, hd=64):
  qkv = x @ W_qkv + b_qkv ; per head: S = q k^T / 8
  A   = exp(S) * mask      (mask = p[key] off-diag, 1 on diag)
  out = (A @ v) / (sum_k A) ; y = out @ W_proj + b_proj
Softmax max-subtraction is skipped (exp args bounded); the ~1e-6-relative
eps regularizers are dropped (denominators are O(100)).

v2 structure (vs the v1 baseline; total rel err ~5.2e-3):
  - weights/biases/policy-derived constants live in the persist pool:
    loaded/computed once per NEFF, shared by every replicated body (the
    per-body DMA is just x in bf16 halves on two queues, plus y out)
  - the whole x / W_qk / W_v / W_proj path is bf16 (PE rate is the same
    as f32r, but DMA and SBUF halve and allow cross-body overlap)
  - the always-keep-diagonal policy fix is applied in PSUM by tiny PE
    matmuls (diag(-8 ln p) @ I accumulated onto the S tile, split into
    base-0 / base-64 variants so each accumulation group keeps a single
    lhsT base partition) instead of 96 DVE scalar_tensor_tensor ops --
    this keeps the S -> exp -> AV chain off the DVE
  - softmax denominators (free via the ones lane in the v layout) are
    partition-broadcast by GPSIMD, reciprocal'd on DVE, and the normalize
    multiply is fused with the PSUM->SBUF out copy (no PE broadcast
    matmuls, no eps)
  - PSUM: qk pool (2 banks) + S pair pool (4) + AV (2) = 8 banks; a
    matmul output must not cross a 2 KB PSUM bank, so every matmul
    writes a 512-f32 (or narrower) region

Hardware gotchas encoded here (from v1, found empirically):
  - float32r matmul operands must come from an f32r-dtype producer
  - gpsimd.partition_broadcast only works from AP base partition 0 to a
    full-partition destination
  - two matmuls with different lhsT base partitions must not share a psum
    accumulation group (device fault)
  - engine ops need 32-aligned base partitions
"""

import sys

if "/opt/trn_rl_repo" not in sys.path:
    sys.path.insert(0, "/opt/trn_rl_repo")

import numpy as np

N, B, C = 1024, 8, 768
NH = 12          # heads
HD = 64          # head dim
P = 128          # partitions
NT = N // P      # 8 token tiles
CT = C // P      # 6 channel tiles
CP = CT // 2     # 3 channel-tile pairs (fp8 DoubleRow)
HP = NH // 2     # 6 head pairs
SCALE = 0.125    # hd**-0.5
# per (tile, pair) v-lane layout: [vA(64) | 1 | 0(63) | vB(64)] (+pad)
# head A stationary window = cols [0:65)   -> psum rows 0:64 out, row 64 denom
# head B stationary window = cols [64:192) -> psum rows 64:128 out; window
#   col 0 is head A's ones lane, so psum row 0 = head B's denominator
VW = 192

_CACHE = {}


def _build(reps=1):
    if reps in _CACHE:
        return _CACHE[reps]

    from contextlib import ExitStack

    import concourse.bass as bass
    import concourse.tile as tile
    from concourse import bacc, mybir
    from concourse.masks import make_identity

    f32 = mybir.dt.float32
    f32r = mybir.dt.float32r
    bf16 = mybir.dt.bfloat16
    fp8 = mybir.dt.float8e4
    Act = mybir.ActivationFunctionType
    DR = mybir.MatmulPerfMode.DoubleRow

    nc = bacc.Bacc()

    xT_d = nc.declare_dram_parameter("xT", [C, N], bf16, isOutput=False)
    pol_d = nc.declare_dram_parameter("policy", [N], f32, isOutput=False)
    wqk_d = nc.declare_dram_parameter("W_qk", [C, 2 * C], bf16, isOutput=False)
    wv_d = nc.declare_dram_parameter("W_v", [C, C], bf16, isOutput=False)
    wproj_d = nc.declare_dram_parameter("Wp16", [C, C], bf16, isOutput=False)
    bqkv_d = nc.declare_dram_parameter("b_qkv", [3 * C], f32, isOutput=False)
    bproj_d = nc.declare_dram_parameter("b_proj", [C], f32, isOutput=False)
    y_d = nc.declare_dram_parameter("y", [N, C], f32, isOutput=True)

    xT_v = xT_d.rearrange("(ct p) n -> p ct n", p=P)           # (128, 6, 1024)
    pol_v = pol_d.rearrange("(t p) -> p t", p=P)               # (128, 8)
    wqk_v = wqk_d.rearrange("(ct p) j -> p ct j", p=P)         # (128, 6, 1536)
    wv_v = wv_d.rearrange("(ct p) j -> p ct j", p=P)           # (128, 6, 768)
    wproj_v = wproj_d.rearrange("(ct p) j -> p ct j", p=P)
    bqk_v = bqkv_d[0 : 2 * C].rearrange("(t p) -> p t", p=P)   # (128, 12)
    y_v = y_d.rearrange("(t p) c -> p t c", p=P)

    def bcast(dram_ap, parts):
        # partition-broadcast read of a 1-D dram slice (step-0 partition dim)
        return bass.AP(
            tensor=dram_ap.tensor, offset=dram_ap.offset, ap=[[0, parts]] + dram_ap.ap
        )

    with tile.TileContext(nc) as tc, ExitStack() as ctx:
        persist = ctx.enter_context(tc.tile_pool(name="persist", bufs=1))
        qkT_pool = ctx.enter_context(tc.tile_pool(name="qkT", bufs=4))
        A_pool = ctx.enter_context(tc.tile_pool(name="Apool", bufs=10))
        rb_pool = ctx.enter_context(tc.tile_pool(name="rbpool", bufs=2))
        d_pool = ctx.enter_context(tc.tile_pool(name="dpool", bufs=2))

        # ---- weights / biases (once per NEFF) ---------------------------
        wqk_sb = persist.tile([P, CT, 2 * C], bf16, name="wqk")    # 18 KB
        # Act's DGE queue: leaves sync free for the first body's x halves
        nc.scalar.dma_start(wqk_sb, wqk_v)
        wv_sb = persist.tile([P, CT, C], bf16, name="wv")          # 9 KB
        nc.gpsimd.dma_start(wv_sb, wv_v)
        wproj_sb = persist.tile([P, CT, C], bf16, name="wproj")    # 9 KB
        nc.gpsimd.dma_start(wproj_sb, wproj_v)
        bqk_sb = persist.tile([P, 2 * CT], f32)
        nc.sync.dma_start(bqk_sb, bqk_v)
        bv_bc = persist.tile([P, C], f32)
        nc.sync.dma_start(bv_bc, bcast(bqkv_d[2 * C : 3 * C], P))
        bproj_bc = persist.tile([P, C], f32)
        nc.sync.dma_start(bproj_bc, bcast(bproj_d[:], P))

        # ---- policy-derived constants (once per NEFF) -------------------
        ident = persist.tile([P, P], f32)
        make_identity(nc, ident)
        identB = persist.tile([P, P], bf16)
        nc.vector.tensor_copy(identB, ident)
        # identSh rows 64:128 = I[0:64]  (for the base-64 diag fix, k < 64)
        identSh = persist.tile([P, P], bf16)
        nc.vector.tensor_copy(identSh[HD:P, :], identB[0:HD, :])

        pol_sb = persist.tile([P, NT], f32)
        nc.sync.dma_start(pol_sb, pol_v)
        logp = persist.tile([P, NT], f32)
        # clamp away exact zeros so ln() stays finite (ln(1e-38) = -87.5)
        nc.vector.tensor_scalar_max(logp, pol_sb, 1e-38)
        nc.scalar.activation(logp, logp, Act.Ln)
        n8logp = persist.tile([P, NT], f32)
        nc.vector.tensor_scalar_mul(n8logp, logp, -8.0)
        # D_full[:, t, :] = diag(-8 ln p) for key tile t (bf16, lhsT base 0)
        D_full = persist.tile([P, NT, P], bf16)
        for t in range(NT):
            nc.vector.tensor_scalar_mul(D_full[:, t, :], identB, n8logp[:, t : t + 1])
        # Dsh: base-64 diag-fix stationaries, rows 64:128:
        #   [:, t, 0:128]   = D_full[0:64, t, :]   (diag cells k < 64)
        #   [:, t, 128:256] = D_full[64:128, t, :] (diag cells k >= 64)
        Dsh = persist.tile([P, NT, 2 * P], bf16)
        for t in range(NT):
            nc.vector.tensor_copy(Dsh[HD:P, t, 0:P], D_full[0:HD, t, :])
            nc.vector.tensor_copy(Dsh[HD:P, t, P : 2 * P], D_full[HD:P, t, :])

        def emit_body(rep):
            body = ExitStack()
            bpool = body.enter_context(tc.tile_pool(name=f"body{rep}", bufs=1))

            # ---- activations in, consumption order ----------------------
            # split x into halves on separate DGE queues: the second half
            # (and the next body's first half) overlaps compute
            xTa = bpool.tile([P, CT // 2, N], bf16, name="xTa")  # 6 KB
            nc.sync.dma_start(xTa, xT_v[:, 0 : CT // 2, :])
            xTb = bpool.tile([P, CT // 2, N], bf16, name="xTb")  # 6 KB
            nc.gpsimd.dma_start(xTb, xT_v[:, CT // 2 : CT, :])

            def xt(ct):
                return xTa[:, ct, :] if ct < CT // 2 else xTb[:, ct - CT // 2, :]

            vv = bpool.tile([P, NT, HP, VW], bf16, name="vv")   # 18 KB
            outT = bpool.tile([P, HP, N], bf16, name="outT")    # 12 KB

            # ones/zeros lanes of the v layout
            nc.gpsimd.memset(vv[:, :, :, HD : 2 * HD], 0.0)
            nc.gpsimd.memset(vv[:, :, :, HD : HD + 1], 1.0)

            # ---- phase 1: v projection ----------------------------------
            with tc.tile_pool(name=f"psv{rep}", bufs=3, space="PSUM") as psv:
                for t in range(NT):
                    pv = psv.tile([P, C], f32, tag="pv")
                    for ct in range(CT):
                        lhs = xt(ct)[:, t * P : (t + 1) * P]
                        nc.tensor.matmul(
                            pv[:, 0:512], lhs, wv_sb[:, ct, 0:512],
                            start=(ct == 0), stop=(ct == CT - 1),
                        )
                        nc.tensor.matmul(
                            pv[:, 512:768], lhs, wv_sb[:, ct, 512:768],
                            start=(ct == 0), stop=(ct == CT - 1),
                        )
                    pv3 = pv.rearrange("p (hp w) -> p hp w", w=P)
                    bv3 = bv_bc.rearrange("p (hp w) -> p hp w", w=P)
                    nc.vector.tensor_add(
                        vv[:, t, :, 0:HD], pv3[:, :, 0:HD], bv3[:, :, 0:HD]
                    )
                    nc.vector.tensor_add(
                        vv[:, t, :, P : P + HD], pv3[:, :, HD:P], bv3[:, :, HD:P]
                    )

            # ---- phase 2: per head-pair: qk -> S(+diag) -> exp -> AV ----
            with tc.tile_pool(
                name=f"psqk{rep}", bufs=1, space="PSUM"
            ) as psqk, tc.tile_pool(
                name=f"ps{rep}", bufs=2, space="PSUM"
            ) as ps, tc.tile_pool(name=f"psav{rep}", bufs=1, space="PSUM") as psav:
                for hp in range(HP):
                    # --- kT, qT for this pair (two 128-row j-tiles) ---
                    # kT first: its copy-out overlaps the qT matmuls; both
                    # copies split in halves so S can start off the first
                    pair_qk = []
                    for sec, jt in ((1, CT + hp), (0, hp)):  # k tile, q tile
                        colbase = sec * C + hp * P
                        pq = psqk.tile([P, N], f32, tag="pq", name=f"pq{sec}")
                        for ct in range(CT):
                            for h in range(2):
                                nc.tensor.matmul(
                                    pq[:, h * 512 : (h + 1) * 512],
                                    wqk_sb[:, ct, colbase : colbase + P],
                                    xt(ct)[:, h * 512 : (h + 1) * 512],
                                    start=(ct == 0), stop=(ct == CT - 1),
                                )
                        dst = qkT_pool.tile([P, N], bf16, tag="qkT")
                        for h in range(2):
                            sl = slice(h * 512, (h + 1) * 512)
                            nc.vector.tensor_scalar_add(
                                dst[:, sl], pq[:, sl], bqk_sb[:, jt : jt + 1]
                            )
                        pair_qk.append(dst)
                    kT_t, qT_t = pair_qk

                    # --- S_T (+ PE diag fix) + masked exp, per key tile ---
                    A_t = [None, None]  # [hi][tk] -> bf16 [128, 1024]
                    A_t[0] = [None] * NT
                    A_t[1] = [None] * NT
                    for tk in range(NT):
                        hdq = tk * P  # column of this tile's diagonal block
                        # interleave the two heads' matmuls so adjacent MMs
                        # sit in disjoint PE row groups (0:64 vs 64:128) and
                        # run concurrently in the array
                        dh = tk // 4  # which 512-half holds the diag block
                        pss_pair = [
                            ps.tile([P, N], f32, tag="s", name=f"ps{i}")
                            for i in range(2)
                        ]
                        for h in range(2):
                            for hi, o in ((0, 0), (1, HD)):
                                nc.tensor.matmul(
                                    pss_pair[hi][:, h * 512 : (h + 1) * 512],
                                    kT_t[o : o + HD, tk * P : (tk + 1) * P],
                                    qT_t[o : o + HD, h * 512 : (h + 1) * 512],
                                    start=True, stop=(h != dh),
                                )
                        # always-keep diagonal: S += diag(-8 ln p) via tiny
                        # PE matmuls; lhsT base must match the S matmul's
                        # (base 0 for head A, base 64 for head B)
                        nc.tensor.matmul(
                            pss_pair[0][:, hdq : hdq + P],
                            D_full[:, tk, :], identB,
                            start=False, stop=True,
                        )
                        nc.tensor.matmul(
                            pss_pair[1][:, hdq : hdq + P],
                            Dsh[HD:P, tk, 0:P], identSh[HD:P, :],
                            start=False, stop=False,
                        )
                        nc.tensor.matmul(
                            pss_pair[1][:, hdq : hdq + P],
                            Dsh[HD:P, tk, P : 2 * P], identB[HD:P, :],
                            start=False, stop=True,
                        )
                        for hi in range(2):
                            pss = pss_pair[hi]
                            At = A_pool.tile([P, N], bf16, tag="A")
                            # A = exp(s/8 + ln p_key); diag pre-biased so the
                            # ln p cancels there (always-keep)
                            nc.scalar.activation(
                                At, pss, Act.Exp,
                                bias=logp[:, tk : tk + 1], scale=SCALE,
                            )
                            A_t[hi][tk] = At

                    # --- AV + denominator + normalize, per head ---
                    for hi in range(2):
                        pav = psav.tile([P, N], f32, tag="av")
                        if hi == 0:
                            vsl = (0, HD + 1)   # [vA | 1] -> rows 0:64, 64=den
                            orows, drow = (0, HD), HD
                        else:
                            vsl = (HD, HD + P)  # -> rows 64:128 out, 0 = den
                            orows, drow = (HD, P), 0
                        mrows = vsl[1] - vsl[0]
                        for tk in range(NT):
                            for h in range(2):
                                nc.tensor.matmul(
                                    pav[:mrows, h * 512 : (h + 1) * 512],
                                    vv[:, tk, hp, vsl[0] : vsl[1]],
                                    A_t[hi][tk][:, h * 512 : (h + 1) * 512],
                                    start=(tk == 0), stop=(tk == NT - 1),
                                )
                        # denominator row -> sbuf -> broadcast -> 1/x
                        dd = d_pool.tile([1, N], f32, tag="d")
                        nc.vector.tensor_copy(dd, pav[drow : drow + 1, :])
                        rb = rb_pool.tile([P, N], f32, tag="rb")
                        nc.gpsimd.partition_broadcast(rb, dd[0:1, :])
                        nc.vector.reciprocal(
                            rb[orows[0] : orows[1], :], rb[orows[0] : orows[1], :]
                        )
                        # normalize fused with the PSUM->SBUF copy
                        nc.vector.tensor_mul(
                            outT[orows[0] : orows[1], hp, :],
                            pav[orows[0] : orows[1], :],
                            rb[orows[0] : orows[1], :],
                        )

            # ---- phase 3: output projection -----------------------------
            # the first two tiles are split partial/finish: their ct 0..4
            # matmuls depend only on head-pairs 0..4, bridging the last
            # pair's normalize latency; the rest stay serial (tail shape
            # unchanged)
            with tc.tile_pool(
                name=f"psy{rep}", bufs=3, space="PSUM"
            ) as psy, tc.tile_pool(name=f"yout{rep}", bufs=4) as yp:
                pys = {}

                def proj_mms(py, t, cts, start, stop):
                    for ct in cts:
                        for sl in (slice(0, 512), slice(512, 768)):
                            nc.tensor.matmul(
                                py[:, sl],
                                outT[:, ct, t * P : (t + 1) * P],
                                wproj_sb[:, ct, sl],
                                start=start and ct == cts[0],
                                stop=stop and ct == cts[-1],
                            )

                def proj_finish(t, py):
                    proj_mms(py, t, [CT - 1], False, True)
                    y_sb = yp.tile([P, C], f32, tag="yo")
                    nc.vector.tensor_add(y_sb, py, bproj_bc)
                    nc.gpsimd.dma_start(y_v[:, t, :], y_sb)

                for t in (0, 1):
                    pys[t] = psy.tile([P, C], f32, tag="y", name=f"py{t}")
                    proj_mms(pys[t], t, list(range(CT - 1)), True, False)
                for t in (0, 1):
                    proj_finish(t, pys.pop(t))
                for t in range(2, NT):
                    py = psy.tile([P, C], f32, tag="y", name=f"py{t}")
                    proj_mms(py, t, list(range(CT - 1)), True, False)
                    proj_finish(t, py)
            body.close()

        for rep in range(reps):
            emit_body(rep)

    nc.finalize()
    _CACHE[reps] = nc
    return nc


def make_in_maps(x, policy, W_qkv, b_qkv, W_proj, b_proj):
    import ml_dtypes

    fp8 = ml_dtypes.float8_e4m3

    x = np.asarray(x, dtype=np.float32)           # (N, B, C)
    policy = np.asarray(policy, dtype=np.float32).reshape(B, N)
    W_qkv = np.asarray(W_qkv, dtype=np.float32)   # (C, 3C)
    W_proj = np.asarray(W_proj, dtype=np.float32)

    bfl = ml_dtypes.bfloat16
    shared = {
        "W_qk": np.ascontiguousarray(W_qkv[:, 0 : 2 * C].astype(bfl)),
        "W_v": np.ascontiguousarray(W_qkv[:, 2 * C : 3 * C].astype(bfl)),
        "Wp16": np.ascontiguousarray(W_proj.astype(ml_dtypes.bfloat16)),
        "b_qkv": np.ascontiguousarray(np.asarray(b_qkv, dtype=np.float32)),
        "b_proj": np.ascontiguousarray(np.asarray(b_proj, dtype=np.float32)),
    }
    maps = []
    for b in range(B):
        xT = np.ascontiguousarray(x[:, b, :].T.astype(bfl))  # (C, N)
        maps.append({
            "xT": xT,
            "policy": np.ascontiguousarray(policy[b]),
            **shared,
        })
    return maps


def kernel(x, policy, W_qkv, b_qkv, W_proj, b_proj):
    from concourse.bass_utils import run_bass_kernel_spmd

    nc = _build()
    in_maps = make_in_maps(x, policy, W_qkv, b_qkv, W_proj, b_proj)
    res = run_bass_kernel_spmd(nc, in_maps, core_ids=list(range(B)))
    y = np.stack([res.results[i]["y"] for i in range(B)], axis=1)  # (N, B, C)
    return np.ascontiguousarray(y.astype(np.float32))
